# revision 6
# baseline (speedup 1.0000x reference)
"""Combined focal + MDCA loss kernel for Trainium2 (8 NeuronCores, SPMD).

reference (jax):
    logp   = log_softmax(logits, axis=1)                 # [B, C]
    logpt  = logp[b, targets[b]]                         # [B]
    pt     = exp(logpt)
    focal  = mean(-(1-pt)^2 * logpt)
    probs  = softmax(logits, axis=1)
    mdca   = mean_c | mean_b probs[b,c] - counts_c / B |
    loss   = focal + 5 * mdca

Strategy (data-parallel over the batch axis, 16384 rows/core, 128 row-tiles
of [128, 1000] each, one batch row per SBUF partition):

  per tile:
    ACT : e = exp(logits)               (accum_out -> s = row-sum of e)
    DVE : me = (iota == target) * e     (scalar_tensor_tensor;
                                         accum_out -> e_t = e[row, target])
    DVE : rs = 1/s ; ret = 1/e_t
    PE  : conf_psum   += rs^T  @ e      (column sums of softmax probs)
          counts_psum += ret^T @ me     (each row contributes e_t/e_t = 1
                                         at its target column)
  finalize (on device, batched over the 128 per-tile stat columns):
    pt = e_t * rs ; logpt = log(pt) ; f = (1-pt)^2 * logpt (+ row reduce)
  host:
    sum the 8 cores' partials in f64, finish the tiny [1000]-vector math.

Max-subtraction is skipped: inputs are standard-normal so exp() is safe in
fp32, and log-sum-exp is algebraically identical with or without the shift.
"""

import numpy as np

import bass_rust
import concourse.bass as bass
import concourse.tile as tile
from concourse import mybir
from concourse.bass_utils import run_bass_kernel_spmd

N_CORES = 8
B, C = 131072, 1000
ROWS = B // N_CORES  # rows per core
P = 128              # partitions (batch rows per tile)
GAMMA = 2.0
BETA = 5.0
NSPLIT = 512         # PSUM bank / matmul free-dim split of C


def _split_excess_waits(nc, max_waits=1):
    """The walrus codegen on this path encodes at most one sync-wait per
    instruction for several instruction structs (Drain/TPB_CTRL, the
    scalar_tensor_tensor S2S2D2_STT, ...), but the Tile scheduler freely
    attaches several. Hoist every wait beyond `max_waits` onto its own
    EventSemaphore instruction placed immediately before the owner, on the
    same engine — semantically identical, since same-engine instructions
    execute in queue order."""
    for bbb in nc.bb_map.values():
        bb = bbb.bb
        insts = list(bb.instructions)
        out = []
        changed = False
        for ins in insts:
            si = ins.sync_info
            if si is not None and len(si.on_wait) > max_waits:
                waits = list(si.on_wait)
                for w in waits[max_waits:]:
                    ev = mybir.InstEventSemaphore(
                        name=nc.get_next_instruction_name(), ins=[], outs=[]
                    )
                    ev.engine = ins.engine
                    ev.sync_info = bass_rust.SyncInfo(on_wait=[w], on_update=[])
                    try:
                        nc.register_instruction(ev)
                    except Exception:
                        pass
                    out.append(ev)
                si.on_wait = waits[:max_waits]
                changed = True
            out.append(ins)
        if changed:
            bb.instructions = out


def build(rows=ROWS, in_bufs=6, use_ttr=False, use_stt=True):
    """Build the per-core Bass module for a `rows`-row shard."""
    nt = rows // P  # number of [P, C] tiles
    f32 = mybir.dt.float32
    AF = mybir.ActivationFunctionType
    OP = mybir.AluOpType

    nc = bass.Bass()
    lg = nc.dram_tensor("logits", [rows, C], f32, kind="ExternalInput")
    tcols = nc.dram_tensor("tcols", [P, nt], f32, kind="ExternalInput")
    out_vec = nc.dram_tensor("out_vec", [1, 2 * C], f32, kind="ExternalOutput")
    out_focal = nc.dram_tensor("focal", [P, 1], f32, kind="ExternalOutput")

    with tile.TileContext(nc) as tc:
        with (
            tc.tile_pool(name="singles", bufs=1) as singles,
            tc.tile_pool(name="inp", bufs=in_bufs) as inp,
            tc.tile_pool(name="ework", bufs=3) as ework,
            tc.tile_pool(name="mework", bufs=3) as mework,
            tc.tile_pool(name="small", bufs=4) as small,
            tc.tile_pool(name="psum", bufs=1, space="PSUM") as psum,
        ):
            iota = singles.tile([P, C], f32)
            nc.gpsimd.iota(
                iota,
                pattern=[[1, C]],
                base=0,
                channel_multiplier=0,
                allow_small_or_imprecise_dtypes=True,
            )
            tcols_sb = singles.tile([P, nt], f32)
            nc.sync.dma_start(out=tcols_sb, in_=tcols[:])

            s_cols = singles.tile([P, nt], f32)    # row-sums of e
            et_cols = singles.tile([P, nt], f32)   # gathered e[row, target]
            rs_cols = singles.tile([P, nt], f32)   # 1/s

            conf_ps = [
                psum.tile([1, NSPLIT], f32, name="conf0"),
                psum.tile([1, C - NSPLIT], f32, name="conf1"),
            ]
            cnt_ps = [
                psum.tile([1, NSPLIT], f32, name="cnt0"),
                psum.tile([1, C - NSPLIT], f32, name="cnt1"),
            ]

            for i in range(nt):
                lt = inp.tile([P, C], f32)
                nc.sync.dma_start(out=lt, in_=lg[i * P : (i + 1) * P, :])

                e = ework.tile([P, C], f32)
                nc.scalar.activation(
                    out=e, in_=lt, func=AF.Exp, accum_out=s_cols[:, i : i + 1]
                )

                me = mework.tile([P, C], f32)
                if use_stt:
                    nc.vector.scalar_tensor_tensor(
                        out=me,
                        in0=iota,
                        scalar=tcols_sb[:, i : i + 1],
                        in1=e,
                        op0=OP.is_equal,
                        op1=OP.mult,
                        accum_out=et_cols[:, i : i + 1],
                    )
                else:
                    mask = mework.tile([P, C], f32, name="mask")
                    nc.vector.tensor_scalar(
                        out=mask,
                        in0=iota,
                        scalar1=tcols_sb[:, i : i + 1],
                        scalar2=None,
                        op0=OP.is_equal,
                    )
                    nc.vector.tensor_tensor(out=me, in0=mask, in1=e, op=OP.mult)
                    nc.vector.tensor_reduce(
                        out=et_cols[:, i : i + 1],
                        in_=me,
                        axis=mybir.AxisListType.X,
                        op=OP.add,
                    )

                rs = rs_cols[:, i : i + 1]
                nc.vector.reciprocal(out=rs, in_=s_cols[:, i : i + 1])
                ret = small.tile([P, 1], f32)
                nc.vector.reciprocal(out=ret, in_=et_cols[:, i : i + 1])

                start, stop = i == 0, i == nt - 1
                nc.tensor.matmul(
                    conf_ps[0], rs, e[:, :NSPLIT], start=start, stop=stop
                )
                nc.tensor.matmul(
                    conf_ps[1], rs, e[:, NSPLIT:], start=start, stop=stop
                )
                nc.tensor.matmul(
                    cnt_ps[0], ret, me[:, :NSPLIT], start=start, stop=stop
                )
                nc.tensor.matmul(
                    cnt_ps[1], ret, me[:, NSPLIT:], start=start, stop=stop
                )

            # ---- focal finalize over the [P, nt] stat arrays ----
            pt = singles.tile([P, nt], f32)
            nc.vector.tensor_tensor(out=pt, in0=et_cols, in1=rs_cols, op=OP.mult)
            logpt = singles.tile([P, nt], f32)
            nc.scalar.activation(out=logpt, in_=pt, func=AF.Ln)
            w = singles.tile([P, nt], f32)
            # (1 - pt)^2 = Square(-1 * pt + 1)
            nc.scalar.activation(out=w, in_=pt, func=AF.Square, bias=1.0, scale=-1.0)
            focal_rows = singles.tile([P, 1], f32)
            fprod = singles.tile([P, nt], f32)
            if use_ttr:
                nc.vector.tensor_tensor_reduce(
                    out=fprod,
                    in0=w,
                    in1=logpt,
                    scale=1.0,
                    scalar=0.0,
                    op0=OP.mult,
                    op1=OP.add,
                    accum_out=focal_rows,
                )
            else:
                nc.vector.tensor_tensor(out=fprod, in0=w, in1=logpt, op=OP.mult)
                nc.vector.tensor_reduce(
                    out=focal_rows, in_=fprod, axis=mybir.AxisListType.X, op=OP.add
                )
            nc.sync.dma_start(out=out_focal[:], in_=focal_rows)

            # ---- conf / counts PSUM -> SBUF -> DRAM ----
            ov = singles.tile([1, 2 * C], f32)
            nc.scalar.copy(out=ov[:, :NSPLIT], in_=conf_ps[0])
            nc.scalar.copy(out=ov[:, NSPLIT:C], in_=conf_ps[1])
            nc.scalar.copy(out=ov[:, C : C + NSPLIT], in_=cnt_ps[0])
            nc.scalar.copy(out=ov[:, C + NSPLIT :], in_=cnt_ps[1])
            nc.sync.dma_start(out=out_vec[:], in_=ov)

    _split_excess_waits(nc)
    return nc


_NC_CACHE = {}


def _get_nc():
    if "nc" not in _NC_CACHE:
        _NC_CACHE["nc"] = build()
    return _NC_CACHE["nc"]


def make_in_maps(logits, targets):
    """Shard full inputs into per-core input maps."""
    logits = np.ascontiguousarray(np.asarray(logits, dtype=np.float32))
    targets = np.asarray(targets).astype(np.int64)
    nt = ROWS // P
    in_maps = []
    for c in range(N_CORES):
        lsh = logits[c * ROWS : (c + 1) * ROWS]
        tsh = targets[c * ROWS : (c + 1) * ROWS]
        # tile i's partition p holds row i*P + p  ->  tcols[p, i]
        tcols = np.ascontiguousarray(tsh.reshape(nt, P).T.astype(np.float32))
        in_maps.append({"logits": lsh, "tcols": tcols})
    return in_maps


def combine(results):
    """Host-side unshard: sum per-core partials, finish the scalar loss."""
    conf = np.zeros(C, np.float64)
    cnt = np.zeros(C, np.float64)
    focal_sum = 0.0
    for r in results:
        v = r["out_vec"][0].astype(np.float64)
        conf += v[:C]
        cnt += v[C:]
        focal_sum += r["focal"].astype(np.float64).sum()
    loss_focal = -focal_sum / B
    loss_mdca = np.abs(conf / B - cnt / B).mean()
    return np.float32(loss_focal + BETA * loss_mdca)


def kernel(logits, targets):
    nc = _get_nc()
    in_maps = make_in_maps(logits, targets)
    res = run_bass_kernel_spmd(nc, in_maps, list(range(N_CORES)))
    return combine(res.results)
